# revision 1
# baseline (speedup 1.0000x reference)
"""Trainium2 Bass kernel for the BEMv13 MoE-LoRA module.

Computation (per token t, full problem):
  base  = x @ W_base.T + b_base
  w     = softmax(x @ W_router + b_router)        # E=2 experts
  H     = x @ A_cat.T                             # [T, 16] LoRA down-proj, both experts
  G     = H * w_broadcast * (alpha/rank)          # per-expert routing weight
  out   = base + G @ B_cat.T

Sharding: tokens (batch*seq = 16384) split evenly across 8 NeuronCores;
all weights replicated. No cross-core communication.

On-core algorithm (per core, 2048 tokens, one 128-token tile at a time):
  - W_base is pre-transposed on host to W^T [D, O]; rounded on-chip to
    float32r (TF32-like) and kept resident in SBUF (128 KB/partition),
    streamed in as half-slabs so arrival matches PE consumption.
  - x arrives token-major; each [128,128] tile is transposed on the PE
    (fp32 transpose) and rounded to float32r in the PSUM->SBUF copy (ACT).
    Tile t's body emits tile t+1's transposes interleaved between matmul
    chunks so the PE stream stays dense.
  - Main matmul: out[128 tok, 512 o] accumulated over 16 k-tiles in PSUM
    (5 rotating banks for the 4 accumulators), float32r operands
    (1 cycle/row, ~1.1e-4 rel err); compiled with walrus LDWEIGHTS dedup.
  - Router logit difference and LoRA H are fused into one small rhs
    (aat, 18 cols) sharing the stationary x^T tile of each k-group.
  - softmax over 2 experts == sigmoid of the logit difference.
  - G^T (PE transpose of the scaled H) is a final K=16 accumulation step
    with B_cat^T into the same PSUM banks; bias is fused into the drain.
  - Dummy "HAM warmer" matmuls in tile 0 keep the PE clock at full rate
    while it trails the W^T DMA stream at startup.
"""

import numpy as np

P = 128
D = 2048
O = 2048
KT = D // P            # 16 k-tiles
TOK = 2048             # tokens per core
HN = 18                # 16 LoRA cols + 1 router-diff col + 1 pad (fp32r needs even N)
ER = 16                # E*R
SCALE = 16.0 / 8.0
NCORES = 8

_CACHE = {}


def _build():
    import concourse.tile as tile
    import concourse.masks as masks
    from concourse import bacc, mybir

    f32 = mybir.dt.float32
    f32r = mybir.dt.float32r

    nc = bacc.Bacc("TRN2", target_bir_lowering=False, debug=False)

    xs_d = nc.dram_tensor("xs", [TOK, D], f32, kind="ExternalInput")
    wt_d = nc.dram_tensor("wt", [D, O], f32, kind="ExternalInput")
    aat_d = nc.dram_tensor("aat", [P, KT * HN], f32, kind="ExternalInput")
    bt_d = nc.dram_tensor("bt", [ER, O], f32, kind="ExternalInput")
    bb_d = nc.dram_tensor("bb", [1, O], f32, kind="ExternalInput")
    brd_d = nc.dram_tensor("brd", [1, 1], f32, kind="ExternalInput")
    out_d = nc.dram_tensor("out", [TOK, O], f32, kind="ExternalOutput")

    with tile.TileContext(nc) as tc:
        with (
            tc.tile_pool(name="res", bufs=1) as res,
            tc.tile_pool(name="big2k", bufs=2) as big2k,
            tc.tile_pool(name="xpool", bufs=2) as xpool,
            tc.tile_pool(name="xtpool", bufs=8) as xtpool,
            tc.tile_pool(name="small", bufs=2) as small,
            tc.tile_pool(name="psA", bufs=5, space="PSUM") as psA,
            tc.tile_pool(name="psT", bufs=2, space="PSUM") as psT,
            tc.tile_pool(name="psH", bufs=1, space="PSUM") as psH,
        ):
            ident = res.tile([P, P], f32, tag="ident")
            masks.make_identity(nc, ident[:])

            # x tok-tile loads go on the ACT HWDGE queue set, W^T streams on
            # the SP set — independent queues, PE transposes start early.
            NT = TOK // P
            x32_tiles = [None] * NT

            def load_x(t, chunks=1):
                x32_tiles[t] = xpool.tile([P, D], f32, tag="x32", name=f"x32_{t}")
                cw = D // chunks
                for cc in range(chunks):
                    nc.scalar.dma_start(
                        x32_tiles[t][:, cc * cw:(cc + 1) * cw],
                        xs_d[t * P:(t + 1) * P, cc * cw:(cc + 1) * cw])

            load_x(0, chunks=4)
            load_x(1, chunks=2)

            # small constants also on the ACT queue set (land in ~us)
            aat32 = res.tile([P, KT * HN], f32, tag="aat32")
            nc.scalar.dma_start(aat32[:], aat_d[:])
            bt32 = res.tile([ER, O], f32, tag="bt32")
            nc.scalar.dma_start(bt32[:], bt_d[:])
            # biases DMA'd directly with partition-broadcast source APs
            bb128 = res.tile([P, O], f32, tag="bb128")
            nc.gpsimd.dma_start(bb128[:], bb_d[:].broadcast_to((P, O)))
            brd128 = res.tile([P, 1], f32, tag="brd128")
            nc.gpsimd.dma_start(brd128[:], brd_d[:].broadcast_to((P, 1)))

            # --- W^T DMAs: SP queue, half-slabs so the arrival granularity
            # (~1.5us) matches the PE's per-k consumption rate.
            HO = O // 2
            wt_r = res.tile([P, KT * O], f32r, tag="wt_r")
            w32s = []
            for k in range(KT):
                w32 = big2k.tile([P, O], f32, tag="big2k", name=f"w32_{k}")
                for hh in range(2):
                    nc.sync.dma_start(w32[:, hh * HO:(hh + 1) * HO],
                                      wt_d[k * P:(k + 1) * P, hh * HO:(hh + 1) * HO])
                w32s.append(w32)

            # Cast order: first two W slabs on DVE (they gate the first
            # matmuls), then small constants on DVE; the remaining W casts go
            # to the otherwise-idle GpSimd engine so the arrival-gated waits
            # don't block tile 0/1's G-phase smalls and drains on the
            # in-order Vector stream.
            def wcast(k, eng=None):
                for hh in range(2):
                    src = w32s[k][:, hh * HO:(hh + 1) * HO]
                    dst = wt_r[:, k * O + hh * HO:k * O + (hh + 1) * HO]
                    if eng is None:
                        nc.scalar.copy(dst, src)
                    else:
                        eng.tensor_copy(dst, src)

            wcast(0, nc.vector)
            wcast(1, nc.vector)
            aat_r = res.tile([P, KT * HN], f32r, tag="aat_r")
            nc.vector.tensor_copy(aat_r[:], aat32[:])
            identr = res.tile([P, P], f32r, tag="identr")
            nc.vector.tensor_copy(identr[:], ident[:])
            bt_r = res.tile([ER, O], f32r, tag="bt_r")
            nc.vector.tensor_copy(bt_r[:], bt32[:])


            # --- main loop: one 128-token tile at a time.
            # Tile t's body also emits tile t+1's PE transposes, interleaved
            # after each 4-k matmul chunk, so the PE stays dense even while
            # trailing the W^T DMA stream at startup.
            xtg_all = [[None] * 4 for _ in range(NT)]

            def emit_transpose_group(t, c):
                st = psT.tile([P, 512], f32, tag="tstage", name=f"ts_{t}_{c}")
                for q in range(4):
                    nc.tensor.transpose(
                        st[:, q * P:(q + 1) * P],
                        x32_tiles[t][:, (c * 4 + q) * P:(c * 4 + q + 1) * P],
                        ident[:],
                    )
                xt = xtpool.tile([P, 512], f32r, tag="xt", name=f"xt_{t}_{c}")
                nc.scalar.copy(xt[:], st[:])
                xtg_all[t][c] = xt

            for c in range(4):
                emit_transpose_group(0, c)
            for c in range(4):
                emit_transpose_group(1, c)

            for t in range(NT):
                if t + 1 < NT and x32_tiles[t + 1] is None:
                    load_x(t + 1)

                accs = [
                    psA.tile([P, 512], f32, tag="acc", name=f"acc_{t}_{j}")
                    for j in range(4)
                ]
                h = psH.tile([P, HN], f32, tag="h", name=f"h_{t}")
                for k in range(KT):
                    lhs = xtg_all[t][k // 4][:, (k % 4) * P:(k % 4 + 1) * P]
                    nc.tensor.matmul(h[:], lhs, aat_r[:, k * HN:(k + 1) * HN],
                                     start=(k == 0), stop=(k == KT - 1))
                    for j in range(4):
                        nc.tensor.matmul(
                            accs[j][:], lhs,
                            wt_r[:, k * O + j * 512:k * O + (j + 1) * 512],
                            start=(k == 0), stop=False,
                        )
                    if k % 4 == 3 and t >= 1 and t + 1 < NT:
                        emit_transpose_group(t + 1, k // 4)
                    # W casts 2..15 ride the ACT stream, interleaved into
                    # tile 0 so arrival-gated waits never block the Vector
                    # stream (tile drains) or the pre-emitted xt copies.
                    if t == 0 and k < 14:
                        wcast(k + 2)
                    # HAM warmers: during the W^T-trailing phase (tiles 0-1)
                    # the PE idles between k-groups and re-throttles to half
                    # clock; re-reading the just-arrived slab keeps it warm.
                    if t == 0:
                        wdum = psT.tile([P, 512], f32, tag="tstage", name=f"wd_{t}_{k}")
                        nc.tensor.matmul(wdum[:], lhs, wt_r[:, k * O:k * O + 512],
                                         start=True, stop=True)

                # routing: w1 = sigmoid(dlogit + brd); scaled by alpha/rank
                srow = small.tile([P, 1], f32, tag="srow", name=f"srow_{t}")
                nc.scalar.activation(srow[:], h[:, ER:ER + 1],
                                     mybir.ActivationFunctionType.Sigmoid,
                                     bias=brd128[:, 0:1], scale=1.0)
                w1s = small.tile([P, 1], f32, tag="w1s", name=f"w1s_{t}")
                nc.vector.tensor_scalar_mul(w1s[:], srow[:], SCALE)
                w0s = small.tile([P, 1], f32, tag="w0s", name=f"w0s_{t}")
                nc.vector.tensor_scalar(w0s[:], srow[:], -SCALE, SCALE,
                                        mybir.AluOpType.mult, mybir.AluOpType.add)
                g = small.tile([P, ER], f32r, tag="g", name=f"g_{t}")
                nc.vector.tensor_scalar_mul(g[:, 0:8], h[:, 0:8], w0s[:])
                nc.vector.tensor_scalar_mul(g[:, 8:16], h[:, 8:16], w1s[:])

                gst = psT.tile([ER, P], f32r, tag="tstage", name=f"gst_{t}")
                nc.tensor.transpose(gst[:], g[:], identr[:])
                gt = small.tile([ER, P], f32r, tag="gt", name=f"gt_{t}")
                nc.vector.tensor_copy(gt[:], gst[:])

                for j in range(4):
                    nc.tensor.matmul(accs[j][:], gt[:],
                                     bt_r[:, j * 512:(j + 1) * 512],
                                     start=False, stop=True)

                outt = big2k.tile([P, O], f32, tag="big2k", name=f"out_{t}")
                for j in range(4):
                    nc.vector.tensor_add(outt[:, j * 512:(j + 1) * 512],
                                         accs[j][:], bb128[:, j * 512:(j + 1) * 512])
                    if j % 2 == 1:
                        nc.sync.dma_start(
                            out_d[t * P:(t + 1) * P, (j - 1) * 512:(j + 1) * 512],
                            outt[:, (j - 1) * 512:(j + 1) * 512])

    nc.compile()
    return nc


def _prep_host(x, W_base, b_base, A, B, W_router, b_router):
    """Host-side layout prep + sharding. Returns per-core input maps."""
    x_flat = np.ascontiguousarray(x, dtype=np.float32).reshape(-1, D)
    wt = np.ascontiguousarray(W_base.T, dtype=np.float32)           # [D, O]
    a_cat = np.asarray(A, dtype=np.float32).reshape(ER, D)          # [16, D]
    aat = np.zeros((D, HN), dtype=np.float32)
    aat[:, :ER] = a_cat.T
    aat[:, ER] = np.asarray(W_router, dtype=np.float32)[:, 1] - np.asarray(W_router, dtype=np.float32)[:, 0]
    # pre-arrange for contiguous per-partition DMA: [P, KT*HN]
    aat = np.ascontiguousarray(aat.reshape(KT, P, HN).transpose(1, 0, 2).reshape(P, KT * HN))
    b_cat = np.concatenate([np.asarray(B, dtype=np.float32)[0],
                            np.asarray(B, dtype=np.float32)[1]], axis=1)  # [O, 16]
    bt = np.ascontiguousarray(b_cat.T)                               # [16, O]
    bb = np.asarray(b_base, dtype=np.float32).reshape(1, O)
    brd = np.array([[np.float32(b_router[1]) - np.float32(b_router[0])]], dtype=np.float32)

    in_maps = []
    for c in range(NCORES):
        in_maps.append({
            "xs": x_flat[c * TOK:(c + 1) * TOK],
            "wt": wt,
            "aat": aat,
            "bt": bt,
            "bb": bb,
            "brd": brd,
        })
    return in_maps


def _enable_ldw_opt():
    """Compile this kernel with walrus's LDWEIGHTS dedup pass. Consecutive
    matmuls here share one stationary operand per k-group; the dedup removes
    the redundant reloads (validated bit-identical output vs. the default)."""
    if _CACHE.get("ldw_patched"):
        return
    import concourse.bass_utils as bu

    orig = bu.run_command

    def patched(argv, **kw):
        argv = [a.replace("--enable-ldw-opt=false", "--enable-ldw-opt=true")
                if isinstance(a, str) else a for a in argv]
        return orig(argv, **kw)

    bu.run_command = patched
    _CACHE["ldw_patched"] = True


def kernel(x, W_base, b_base, A, B, W_router, b_router):
    from concourse import bass_utils

    _enable_ldw_opt()
    if "nc" not in _CACHE:
        _CACHE["nc"] = _build()
    nc = _CACHE["nc"]

    in_maps = _prep_host(x, W_base, b_base, A, B, W_router, b_router)
    res = None
    for attempt in range(3):
        try:
            res = bass_utils.run_bass_kernel_spmd(
                nc, in_maps, core_ids=list(range(NCORES)))
            break
        except Exception:
            # rare transient NRT_EXEC_UNIT_UNRECOVERABLE observed once;
            # the same NEFF runs fine on retry
            if attempt == 2:
                raise
    out = np.concatenate([res.results[c]["out"] for c in range(NCORES)], axis=0)
    return out.reshape(np.asarray(x).shape[0], -1, O)



# revision 5
# speedup vs baseline: 1.3400x; 1.3400x over previous
"""Trainium2 Bass kernel for the BEMv13 MoE-LoRA module.

Computation (per token t, full problem):
  base  = x @ W_base.T + b_base
  w     = softmax(x @ W_router + b_router)        # E=2 experts
  H     = x @ A_cat.T                             # [T, 16] LoRA down-proj, both experts
  G     = H * w_broadcast * (alpha/rank)          # per-expert routing weight
  out   = base + G @ B_cat.T

Sharding: tokens (batch*seq = 16384) split evenly across 8 NeuronCores;
all weights replicated. No cross-core communication.

On-core algorithm (per core, 2048 tokens, one 128-token tile at a time),
all matmul operands in bf16 (abs tolerance is ~1e-1; bf16 error ~1e-2):
  - x is pre-transposed AND pre-tiled on the host to [tile, d, tok] bf16
    so no on-chip transposes or casts are needed at all.
  - W^T [D, O] bf16 is streamed in once and stays resident in SBUF
    (64 KB/partition).
  - Main matmul: out[128 tok, 512 o] accumulated over 16 k-tiles in PSUM
    (5 rotating banks for the 4 accumulators); stationary xt tile shared
    by the LoRA/router matmul (aat, 18 cols) of each k-group.
  - softmax over 2 experts == sigmoid of the logit difference.
  - G is extended with a ones column; B_cat^T with a b_base row, so the
    K=17 LoRA finalize matmul also adds the output bias -> the drain is a
    pure PSUM->SBUF bf16 copy, split between Vector and Scalar engines.
  - Output is written to HBM in bf16 and upcast on the host.
"""

import numpy as np
import ml_dtypes

BF16 = ml_dtypes.bfloat16

P = 128
D = 2048
O = 2048
KT = D // P            # 16 k-tiles
NT = 16                # 128-token tiles per core
TOK = 2048             # tokens per core
HN = 18                # 16 LoRA cols + 1 router-diff col + 1 pad
ER = 16                # E*R
GK = 18                # LoRA finalize: 16 lora rows + 1 bias/ones row + 1 pad
SCALE = 16.0 / 8.0
NCORES = 8

_CACHE = {}


def _build():
    import concourse.tile as tile
    import concourse.masks as masks
    from concourse import bacc, mybir

    f32 = mybir.dt.float32
    bf16 = mybir.dt.bfloat16

    nc = bacc.Bacc("TRN2", target_bir_lowering=False, debug=False)

    # xt: host-pretransposed x, row block t*128+p holds, for col k*128+j,
    # the value x[t*128+j, k*128+p]  (d-major within each token tile)
    xt_d = nc.dram_tensor("xt", [NT * P, KT * P], bf16, kind="ExternalInput")
    wt_d = nc.dram_tensor("wt", [D, O], bf16, kind="ExternalInput")
    aat_d = nc.dram_tensor("aat", [P, KT * HN], bf16, kind="ExternalInput")
    btx_d = nc.dram_tensor("btx", [GK, O], bf16, kind="ExternalInput")
    brd_d = nc.dram_tensor("brd", [1, 1], f32, kind="ExternalInput")
    out_d = nc.dram_tensor("out", [TOK, O], bf16, kind="ExternalOutput")

    with tile.TileContext(nc) as tc:
        with (
            tc.tile_pool(name="res", bufs=1) as res,
            tc.tile_pool(name="xpool", bufs=4) as xpool,
            tc.tile_pool(name="opool", bufs=2) as opool,
            tc.tile_pool(name="gpool", bufs=2) as gpool,
            tc.tile_pool(name="small", bufs=2) as small,
            tc.tile_pool(name="psA", bufs=5, space="PSUM") as psA,
            tc.tile_pool(name="psH", bufs=2, space="PSUM") as psH,
            tc.tile_pool(name="psT", bufs=1, space="PSUM") as psT,
        ):
            # small constants on the ACT HWDGE queue (land in ~us)
            aat_sb = res.tile([P, KT * HN], bf16, tag="aat")
            nc.scalar.dma_start(aat_sb[:], aat_d[:])
            btx_sb = res.tile([GK, O], bf16, tag="btx")
            nc.scalar.dma_start(btx_sb[:], btx_d[:])
            brd128 = res.tile([P, 1], f32, tag="brd128")
            nc.gpsimd.dma_start(brd128[:], brd_d[:].broadcast_to((P, 1)))

            ident = res.tile([P, P], bf16, tag="ident")
            masks.make_identity(nc, ident[:])

            # x token-tile loads ride the ACT HWDGE queue set; W^T streams on
            # the SP set - independent queues.
            xts = [None] * NT

            def load_x(t):
                xts[t] = xpool.tile([P, KT * P], bf16, tag="xt", name=f"xt_{t}")
                nc.scalar.dma_start(xts[t][:], xt_d[t * P:(t + 1) * P, :])

            for t in range(3):
                load_x(t)

            # W^T stream: SP queue, half-slabs so arrival granularity matches
            # PE consumption during the trailing phase (tile 0).
            HO = O // 2
            w_sb = res.tile([P, KT * O], bf16, tag="w_sb")
            for k in range(KT):
                for hh in range(2):
                    nc.sync.dma_start(
                        w_sb[:, k * O + hh * HO:k * O + (hh + 1) * HO],
                        wt_d[k * P:(k + 1) * P, hh * HO:(hh + 1) * HO])

            # --- main loop: one 128-token tile at a time.
            # LoRA finalize for tile t is emitted after tile t's main sweep;
            # the h->sigmoid->g->g^T chain latency is mostly hidden because
            # the h matmul leads each k-group and the chain engines (ACT/DVE)
            # are otherwise idle.
            for t in range(NT):
                if t + 3 < NT:
                    load_x(t + 3)

                accs = [
                    psA.tile([P, 512], f32, tag="acc", name=f"acc_{t}_{j}")
                    for j in range(4)
                ]
                h = psH.tile([P, HN], f32, tag="h", name=f"h_{t}")
                for k in range(KT):
                    lhs = xts[t][:, k * P:(k + 1) * P]
                    nc.tensor.matmul(h[:], lhs, aat_sb[:, k * HN:(k + 1) * HN],
                                     start=(k == 0), stop=(k == KT - 1))
                    for j in range(4):
                        nc.tensor.matmul(
                            accs[j][:], lhs,
                            w_sb[:, k * O + j * 512:k * O + (j + 1) * 512],
                            start=(k == 0), stop=False,
                        )
                    # HAM warmers: while the PE trails the W^T DMA stream in
                    # tile 0 it would idle between k-groups and re-throttle to
                    # half clock; re-reading the just-arrived slab keeps it
                    # warm and costs nothing once W is resident.
                    if t == 0:
                        wdum = psT.tile([P, 512], f32, tag="tstage",
                                        name=f"wd_{k}")
                        nc.tensor.matmul(wdum[:], lhs,
                                         w_sb[:, k * O:k * O + 512],
                                         start=True, stop=True)

                # routing: w1 = sigmoid(dlogit + brd); scaled by alpha/rank
                srow = small.tile([P, 1], f32, tag="srow", name=f"srow_{t}")
                nc.scalar.activation(srow[:], h[:, ER:ER + 1],
                                     mybir.ActivationFunctionType.Sigmoid,
                                     bias=brd128[:, 0:1], scale=1.0)
                w1s = small.tile([P, 1], f32, tag="w1s", name=f"w1s_{t}")
                nc.vector.tensor_scalar_mul(w1s[:], srow[:], SCALE)
                w0s = small.tile([P, 1], f32, tag="w0s", name=f"w0s_{t}")
                nc.vector.tensor_scalar(w0s[:], srow[:], -SCALE, SCALE,
                                        mybir.AluOpType.mult, mybir.AluOpType.add)
                g = gpool.tile([P, GK], bf16, tag="g", name=f"g_{t}")
                nc.vector.tensor_scalar_mul(g[:, 0:8], h[:, 0:8], w0s[:])
                nc.vector.tensor_scalar_mul(g[:, 8:16], h[:, 8:16], w1s[:])
                nc.vector.memset(g[:, 16:17], 1.0)
                nc.vector.memset(g[:, 17:18], 0.0)

                gst = psT.tile([GK, P], bf16, tag="tstage", name=f"gst_{t}")
                nc.tensor.transpose(gst[:], g[:], ident[:])
                gt = gpool.tile([GK, P], bf16, tag="gt", name=f"gt_{t}")
                nc.scalar.copy(gt[:], gst[:])

                for j in range(4):
                    nc.tensor.matmul(accs[j][:], gt[:],
                                     btx_sb[:, j * 512:(j + 1) * 512],
                                     start=False, stop=True)

                # drain: bias already folded in via the ones-row; pure copy,
                # split across Vector and Scalar engines.
                outt = opool.tile([P, O], bf16, tag="outt", name=f"out_{t}")
                for j in range(4):
                    if j < 2:
                        nc.vector.tensor_copy(outt[:, j * 512:(j + 1) * 512],
                                              accs[j][:])
                    else:
                        nc.scalar.copy(outt[:, j * 512:(j + 1) * 512],
                                       accs[j][:])
                    if j % 2 == 1:
                        nc.sync.dma_start(
                            out_d[t * P:(t + 1) * P, (j - 1) * 512:(j + 1) * 512],
                            outt[:, (j - 1) * 512:(j + 1) * 512])

    nc.compile()
    return nc


def _prep_host(x, W_base, b_base, A, B, W_router, b_router):
    """Host-side layout prep + sharding. Returns per-core input maps."""
    x_flat = np.ascontiguousarray(np.asarray(x, dtype=np.float32).reshape(-1, D))
    xb = x_flat.astype(BF16)
    # per-core pre-transposed tiling: [core, t, p(d within k), k*128+j(tok)]
    xt_all = np.ascontiguousarray(
        xb.reshape(NCORES, NT, P, KT, P).transpose(0, 1, 4, 3, 2)
    ).reshape(NCORES, NT * P, KT * P)

    wt = np.ascontiguousarray(np.asarray(W_base, dtype=np.float32).T).astype(BF16)

    a_cat = np.asarray(A, dtype=np.float32).reshape(ER, D)          # [16, D]
    aat = np.zeros((D, HN), dtype=np.float32)
    aat[:, :ER] = a_cat.T
    wr = np.asarray(W_router, dtype=np.float32)
    aat[:, ER] = wr[:, 1] - wr[:, 0]
    # pre-arrange for contiguous per-partition DMA: [P, KT*HN]
    aat = np.ascontiguousarray(
        aat.reshape(KT, P, HN).transpose(1, 0, 2).reshape(P, KT * HN)
    ).astype(BF16)

    b_cat = np.concatenate([np.asarray(B, dtype=np.float32)[0],
                            np.asarray(B, dtype=np.float32)[1]], axis=1)  # [O, 16]
    btx = np.zeros((GK, O), dtype=np.float32)
    btx[:ER] = b_cat.T
    btx[ER] = np.asarray(b_base, dtype=np.float32)   # ones-row in g^T adds bias
    btx = np.ascontiguousarray(btx).astype(BF16)

    brd = np.array([[np.float32(b_router[1]) - np.float32(b_router[0])]],
                   dtype=np.float32)

    in_maps = []
    for c in range(NCORES):
        in_maps.append({
            "xt": xt_all[c],
            "wt": wt,
            "aat": aat,
            "btx": btx,
            "brd": brd,
        })
    return in_maps


def _enable_ldw_opt():
    """Compile this kernel with walrus's LDWEIGHTS dedup pass. Consecutive
    matmuls here share one stationary operand per k-group; the dedup removes
    the redundant reloads (validated bit-identical output vs. the default)."""
    if _CACHE.get("ldw_patched"):
        return
    import concourse.bass_utils as bu

    orig = bu.run_command

    def patched(argv, **kw):
        argv = [a.replace("--enable-ldw-opt=false", "--enable-ldw-opt=true")
                if isinstance(a, str) else a for a in argv]
        return orig(argv, **kw)

    bu.run_command = patched
    _CACHE["ldw_patched"] = True


def kernel(x, W_base, b_base, A, B, W_router, b_router):
    from concourse import bass_utils

    # NOTE: walrus's LDWEIGHTS-dedup (--enable-ldw-opt=true) rejects bf16
    # weight loads ("InstLdweights is not compatible with LDW optimization")
    # because they take the FWL path; bf16 LDWs are ~2x faster than fp32 and
    # hidden by the PE's pull-ahead window, so the dedup isn't needed.
    if "nc" not in _CACHE:
        _CACHE["nc"] = _build()
    nc = _CACHE["nc"]

    in_maps = _prep_host(x, W_base, b_base, A, B, W_router, b_router)
    res = None
    for attempt in range(3):
        try:
            res = bass_utils.run_bass_kernel_spmd(
                nc, in_maps, core_ids=list(range(NCORES)))
            break
        except Exception:
            # rare transient NRT_EXEC_UNIT_UNRECOVERABLE observed once;
            # the same NEFF runs fine on retry
            if attempt == 2:
                raise
    out = np.concatenate(
        [np.asarray(res.results[c]["out"]).astype(np.float32)
         for c in range(NCORES)], axis=0)
    return out.reshape(np.asarray(x).shape[0], -1, O)


# revision 6
# speedup vs baseline: 1.3972x; 1.0426x over previous
"""Trainium2 Bass kernel for the BEMv13 MoE-LoRA module.

Computation (per token t, full problem):
  base  = x @ W_base.T + b_base
  w     = softmax(x @ W_router + b_router)        # E=2 experts
  H     = x @ A_cat.T                             # [T, 16] LoRA down-proj, both experts
  G     = H * w_broadcast * (alpha/rank)          # per-expert routing weight
  out   = base + G @ B_cat.T

Sharding: tokens (batch*seq = 16384) split evenly across 8 NeuronCores;
all weights replicated. No cross-core communication.

On-core algorithm (per core, 2048 tokens, one 128-token tile at a time),
all matmul operands in bf16 (abs tolerance is ~1e-1; bf16 error ~1e-2):
  - x is pre-transposed AND pre-tiled on the host to [tile, d, tok] bf16
    so no on-chip transposes or casts are needed at all.
  - W^T [D, O] bf16 is streamed in once and stays resident in SBUF
    (64 KB/partition).
  - Main matmul: out[128 tok, 512 o] accumulated over 16 k-tiles in PSUM
    (5 rotating banks for the 4 accumulators); stationary xt tile shared
    by the LoRA/router matmul (aat, 18 cols) of each k-group.
  - softmax over 2 experts == sigmoid of the logit difference.
  - G is extended with a ones column; B_cat^T with a b_base row, so the
    K=17 LoRA finalize matmul also adds the output bias -> the drain is a
    pure PSUM->SBUF bf16 copy, split between Vector and Scalar engines.
  - Output is written to HBM in bf16 and upcast on the host.
"""

import numpy as np
import ml_dtypes

BF16 = ml_dtypes.bfloat16

P = 128
D = 2048
O = 2048
KT = D // P            # 16 k-tiles
NT = 16                # 128-token tiles per core
TOK = 2048             # tokens per core
HN = 18                # 16 LoRA cols + 1 router-diff col + 1 pad
ER = 16                # E*R
GK = 18                # LoRA finalize: 16 lora rows + 1 bias/ones row + 1 pad
SCALE = 16.0 / 8.0
NCORES = 8

_CACHE = {}


def _build():
    import concourse.tile as tile
    import concourse.masks as masks
    from concourse import bacc, mybir

    f32 = mybir.dt.float32
    bf16 = mybir.dt.bfloat16

    nc = bacc.Bacc("TRN2", target_bir_lowering=False, debug=False)

    # xt: host-pretransposed x, row block t*128+p holds, for col k*128+j,
    # the value x[t*128+j, k*128+p]  (d-major within each token tile)
    xt_d = nc.dram_tensor("xt", [NT * P, KT * P], bf16, kind="ExternalInput")
    wt_d = nc.dram_tensor("wt", [D, O], bf16, kind="ExternalInput")
    aat_d = nc.dram_tensor("aat", [P, KT * HN], bf16, kind="ExternalInput")
    btx_d = nc.dram_tensor("btx", [P, 512], bf16, kind="ExternalInput")
    brd_d = nc.dram_tensor("brd", [1, 1], f32, kind="ExternalInput")
    out_d = nc.dram_tensor("out", [TOK, O], bf16, kind="ExternalOutput")

    with tile.TileContext(nc) as tc:
        with (
            tc.tile_pool(name="res", bufs=1) as res,
            tc.tile_pool(name="xpool", bufs=4) as xpool,
            tc.tile_pool(name="opool", bufs=2) as opool,
            tc.tile_pool(name="gpool", bufs=2) as gpool,
            tc.tile_pool(name="small", bufs=2) as small,
            tc.tile_pool(name="psA", bufs=5, space="PSUM") as psA,
            tc.tile_pool(name="psH", bufs=2, space="PSUM") as psH,
            tc.tile_pool(name="psT", bufs=1, space="PSUM") as psT,
        ):
            # small constants on the ACT HWDGE queue (land in ~us)
            aat_sb = res.tile([P, KT * HN], bf16, tag="aat")
            nc.scalar.dma_start(aat_sb[:], aat_d[:])
            btx_sb = res.tile([P, 512], bf16, tag="btx")
            nc.scalar.dma_start(btx_sb[:], btx_d[:])
            brd128 = res.tile([P, 1], f32, tag="brd128")
            nc.gpsimd.dma_start(brd128[:], brd_d[:].broadcast_to((P, 1)))

            ident = res.tile([P, P], bf16, tag="ident")
            masks.make_identity(nc, ident[:])

            # x token-tile loads ride the ACT HWDGE queue set; W^T streams on
            # the SP set - independent queues.
            xts = [None] * NT

            def load_x(t):
                xts[t] = xpool.tile([P, KT * P], bf16, tag="xt", name=f"xt_{t}")
                nc.scalar.dma_start(xts[t][:], xt_d[t * P:(t + 1) * P, :])

            for t in range(3):
                load_x(t)

            # W^T stream: SP queue, half-slabs so arrival granularity matches
            # PE consumption during the trailing phase (tile 0).
            HO = O // 2
            w_sb = res.tile([P, KT * O], bf16, tag="w_sb")
            for k in range(KT):
                for hh in range(2):
                    nc.sync.dma_start(
                        w_sb[:, k * O + hh * HO:k * O + (hh + 1) * HO],
                        wt_d[k * P:(k + 1) * P, hh * HO:(hh + 1) * HO])

            # --- main loop: one 128-token tile at a time.
            # LoRA finalize for tile t is emitted after tile t's main sweep;
            # the h->sigmoid->g->g^T chain latency is mostly hidden because
            # the h matmul leads each k-group and the chain engines (ACT/DVE)
            # are otherwise idle.
            for t in range(NT):
                if t + 3 < NT:
                    load_x(t + 3)

                accs = [
                    psA.tile([P, 512], f32, tag="acc", name=f"acc_{t}_{j}")
                    for j in range(4)
                ]
                h = psH.tile([P, HN], f32, tag="h", name=f"h_{t}")
                gt4 = gpool.tile([P, P], bf16, tag="gt4", name=f"gt4_{t}")
                for k in range(KT):
                    lhs = xts[t][:, k * P:(k + 1) * P]
                    # h matmuls front-loaded 2-per-iteration so the routing
                    # chain (sigmoid -> g -> g^T -> gt copies) finishes while
                    # the main sweep still streams.
                    if k < 8:
                        for c in (2 * k, 2 * k + 1):
                            nc.tensor.matmul(
                                h[:], xts[t][:, c * P:(c + 1) * P],
                                aat_sb[:, c * HN:(c + 1) * HN],
                                start=(c == 0), stop=(c == KT - 1))
                    for j in range(4):
                        nc.tensor.matmul(
                            accs[j][:], lhs,
                            w_sb[:, k * O + j * 512:k * O + (j + 1) * 512],
                            start=(k == 0), stop=False,
                        )
                    # HAM warmers: while the PE trails the W^T DMA stream in
                    # tile 0 it would idle between k-groups and re-throttle to
                    # half clock; re-reading the just-arrived slab keeps it
                    # warm and costs nothing once W is resident.
                    if t == 0:
                        wdum = psT.tile([P, 512], f32, tag="tstage",
                                        name=f"wd_{k}")
                        nc.tensor.matmul(wdum[:], lhs,
                                         w_sb[:, k * O:k * O + 512],
                                         start=True, stop=True)
                    if k == 7:
                        # routing: w1 = sigmoid(dlogit + brd), w0 = 1 - w1,
                        # both scaled by alpha/rank
                        srow = small.tile([P, 1], f32, tag="srow",
                                          name=f"srow_{t}")
                        nc.scalar.activation(
                            srow[:], h[:, ER:ER + 1],
                            mybir.ActivationFunctionType.Sigmoid,
                            bias=brd128[:, 0:1], scale=1.0)
                        w1s = small.tile([P, 1], f32, tag="w1s", name=f"w1s_{t}")
                        nc.vector.tensor_scalar_mul(w1s[:], srow[:], SCALE)
                        w0s = small.tile([P, 1], f32, tag="w0s", name=f"w0s_{t}")
                        nc.vector.tensor_scalar(
                            w0s[:], srow[:], -SCALE, SCALE,
                            mybir.AluOpType.mult, mybir.AluOpType.add)
                        g = gpool.tile([P, GK], bf16, tag="g", name=f"g_{t}")
                        nc.vector.tensor_scalar_mul(g[:, 0:8], h[:, 0:8], w0s[:])
                        nc.vector.tensor_scalar_mul(g[:, 8:16], h[:, 8:16], w1s[:])
                        nc.vector.memset(g[:, 16:17], 1.0)
                        nc.vector.memset(g[:, 17:18], 0.0)
                    if k == 11:
                        # g^T on the PE, then replicate into 4 row strips so
                        # the finalize matmuls can run on 4 concurrent 32-row
                        # tiles of the array.
                        gst = psT.tile([GK, P], bf16, tag="tstage",
                                       name=f"gst_{t}")
                        nc.tensor.transpose(gst[:], g[:], ident[:])
                        for jj in range(4):
                            nc.scalar.copy(gt4[32 * jj:32 * jj + GK, :], gst[:])

                # LoRA finalize + bias: 4 concurrent row-tiled K=18 matmuls
                for jj in range(4):
                    nc.tensor.matmul(accs[jj][:],
                                     gt4[32 * jj:32 * jj + GK, :],
                                     btx_sb[32 * jj:32 * jj + GK, :],
                                     start=False, stop=True,
                                     tile_position=(32 * jj, 0))

                # drain: bias already folded in via the ones-row; pure copy,
                # split across Vector and Scalar engines.
                outt = opool.tile([P, O], bf16, tag="outt", name=f"out_{t}")
                for j in range(4):
                    if j < 2:
                        nc.vector.tensor_copy(outt[:, j * 512:(j + 1) * 512],
                                              accs[j][:])
                    else:
                        nc.scalar.copy(outt[:, j * 512:(j + 1) * 512],
                                       accs[j][:])
                    if j % 2 == 1:
                        nc.sync.dma_start(
                            out_d[t * P:(t + 1) * P, (j - 1) * 512:(j + 1) * 512],
                            outt[:, (j - 1) * 512:(j + 1) * 512])

    nc.compile()
    return nc


def _prep_host(x, W_base, b_base, A, B, W_router, b_router):
    """Host-side layout prep + sharding. Returns per-core input maps."""
    x_flat = np.ascontiguousarray(np.asarray(x, dtype=np.float32).reshape(-1, D))
    xb = x_flat.astype(BF16)
    # per-core pre-transposed tiling: [core, t, p(d within k), k*128+j(tok)]
    xt_all = np.ascontiguousarray(
        xb.reshape(NCORES, NT, P, KT, P).transpose(0, 1, 4, 3, 2)
    ).reshape(NCORES, NT * P, KT * P)

    wt = np.ascontiguousarray(np.asarray(W_base, dtype=np.float32).T).astype(BF16)

    a_cat = np.asarray(A, dtype=np.float32).reshape(ER, D)          # [16, D]
    aat = np.zeros((D, HN), dtype=np.float32)
    aat[:, :ER] = a_cat.T
    wr = np.asarray(W_router, dtype=np.float32)
    aat[:, ER] = wr[:, 1] - wr[:, 0]
    # pre-arrange for contiguous per-partition DMA: [P, KT*HN]
    aat = np.ascontiguousarray(
        aat.reshape(KT, P, HN).transpose(1, 0, 2).reshape(P, KT * HN)
    ).astype(BF16)

    b_cat = np.concatenate([np.asarray(B, dtype=np.float32)[0],
                            np.asarray(B, dtype=np.float32)[1]], axis=1)  # [O, 16]
    btx_full = np.zeros((GK, O), dtype=np.float32)
    btx_full[:ER] = b_cat.T
    btx_full[ER] = np.asarray(b_base, dtype=np.float32)  # ones-row adds bias
    # 4 row strips at partition offsets 0/32/64/96, one 512-col chunk each,
    # so the finalize matmuls run on 4 concurrent 32-row PE tiles
    btx = np.zeros((P, 512), dtype=np.float32)
    for jj in range(4):
        btx[32 * jj:32 * jj + GK] = btx_full[:, jj * 512:(jj + 1) * 512]
    btx = np.ascontiguousarray(btx).astype(BF16)

    brd = np.array([[np.float32(b_router[1]) - np.float32(b_router[0])]],
                   dtype=np.float32)

    in_maps = []
    for c in range(NCORES):
        in_maps.append({
            "xt": xt_all[c],
            "wt": wt,
            "aat": aat,
            "btx": btx,
            "brd": brd,
        })
    return in_maps


def kernel(x, W_base, b_base, A, B, W_router, b_router):
    from concourse import bass_utils

    # NOTE: walrus's LDWEIGHTS-dedup (--enable-ldw-opt=true) rejects bf16
    # weight loads ("InstLdweights is not compatible with LDW optimization")
    # because they take the FWL path; bf16 LDWs are ~2x faster than fp32 and
    # hidden by the PE's pull-ahead window, so the dedup isn't needed.
    if "nc" not in _CACHE:
        _CACHE["nc"] = _build()
    nc = _CACHE["nc"]

    in_maps = _prep_host(x, W_base, b_base, A, B, W_router, b_router)
    res = None
    for attempt in range(3):
        try:
            res = bass_utils.run_bass_kernel_spmd(
                nc, in_maps, core_ids=list(range(NCORES)))
            break
        except Exception:
            # rare transient NRT_EXEC_UNIT_UNRECOVERABLE observed once;
            # the same NEFF runs fine on retry
            if attempt == 2:
                raise
    out = np.concatenate(
        [np.asarray(res.results[c]["out"]).astype(np.float32)
         for c in range(NCORES)], axis=0)
    return out.reshape(np.asarray(x).shape[0], -1, O)


# revision 7
# speedup vs baseline: 1.4229x; 1.0184x over previous
"""Trainium2 Bass kernel for the BEMv13 MoE-LoRA module.

Computation (per token t, full problem):
  base  = x @ W_base.T + b_base
  w     = softmax(x @ W_router + b_router)        # E=2 experts
  H     = x @ A_cat.T                             # [T, 16] LoRA down-proj, both experts
  G     = H * w_broadcast * (alpha/rank)          # per-expert routing weight
  out   = base + G @ B_cat.T

Sharding: tokens (batch*seq = 16384) split evenly across 8 NeuronCores;
all weights replicated. No cross-core communication.

On-core algorithm (per core, 2048 tokens, one 128-token tile at a time),
all matmul operands in bf16 (abs tolerance is ~1e-1; bf16 error ~1e-2):
  - x is pre-transposed AND pre-tiled on the host to [tile, d, tok] bf16
    so no on-chip transposes or casts are needed at all.
  - W^T [D, O] bf16 is streamed in once and stays resident in SBUF
    (64 KB/partition).
  - Main matmul: out[128 tok, 512 o] accumulated over 16 k-tiles in PSUM
    (5 rotating banks for the 4 accumulators); stationary xt tile shared
    by the LoRA/router matmul (aat, 18 cols) of each k-group.
  - softmax over 2 experts == sigmoid of the logit difference.
  - G is extended with a ones column; B_cat^T with a b_base row, so the
    K=17 LoRA finalize matmul also adds the output bias -> the drain is a
    pure PSUM->SBUF bf16 copy, split between Vector and Scalar engines.
  - Output is written to HBM in bf16 and upcast on the host.
"""

import numpy as np
import ml_dtypes

BF16 = ml_dtypes.bfloat16

P = 128
D = 2048
O = 2048
KT = D // P            # 16 k-tiles
NT = 16                # 128-token tiles per core
TOK = 2048             # tokens per core
HN = 18                # 16 LoRA cols + 1 router-diff col + 1 pad
ER = 16                # E*R
GK = 18                # LoRA finalize: 16 lora rows + 1 bias/ones row + 1 pad
SCALE = 16.0 / 8.0
NCORES = 8

_CACHE = {}


def _build():
    import concourse.tile as tile
    import concourse.masks as masks
    from concourse import bacc, mybir

    f32 = mybir.dt.float32
    bf16 = mybir.dt.bfloat16

    nc = bacc.Bacc("TRN2", target_bir_lowering=False, debug=False)

    # xt: host-pretransposed x, row block t*128+p holds, for col k*128+j,
    # the value x[t*128+j, k*128+p]  (d-major within each token tile)
    xt_d = nc.dram_tensor("xt", [NT * P, KT * P], bf16, kind="ExternalInput")
    wt_d = nc.dram_tensor("wt", [D, O], bf16, kind="ExternalInput")
    aat_d = nc.dram_tensor("aat", [P, KT * HN], bf16, kind="ExternalInput")
    btx_d = nc.dram_tensor("btx", [P, 512], bf16, kind="ExternalInput")
    brd_d = nc.dram_tensor("brd", [1, 1], f32, kind="ExternalInput")
    out_d = nc.dram_tensor("out", [TOK, O], bf16, kind="ExternalOutput")

    with tile.TileContext(nc) as tc:
        with (
            tc.tile_pool(name="res", bufs=1) as res,
            tc.tile_pool(name="xpool", bufs=4) as xpool,
            tc.tile_pool(name="opool", bufs=2) as opool,
            tc.tile_pool(name="gpool", bufs=2) as gpool,
            tc.tile_pool(name="small", bufs=2) as small,
            tc.tile_pool(name="psA", bufs=6, space="PSUM") as psA,
            tc.tile_pool(name="psH", bufs=1, space="PSUM") as psH,
            tc.tile_pool(name="psT", bufs=1, space="PSUM") as psT,
        ):
            # DMA emission order is engine program order: the first matmul
            # only needs aat + xt0, so those go first on the ACT queue;
            # btx/xt1/xt2 triggers are deferred into tile 0's body so the
            # coalesced DMA-completion semaphore doesn't make the first
            # matmul wait for them.
            aat_sb = res.tile([P, KT * HN], bf16, tag="aat")
            nc.scalar.dma_start(aat_sb[:], aat_d[:])

            xts = [None] * NT

            def load_x(t, chunks=1):
                xts[t] = xpool.tile([P, KT * P], bf16, tag="xt", name=f"xt_{t}")
                cw = KT * P // chunks
                for cc in range(chunks):
                    nc.scalar.dma_start(
                        xts[t][:, cc * cw:(cc + 1) * cw],
                        xt_d[t * P:(t + 1) * P, cc * cw:(cc + 1) * cw])

            load_x(0, chunks=2)

            btx_sb = res.tile([P, 512], bf16, tag="btx")
            brd128 = res.tile([P, 1], f32, tag="brd128")
            nc.gpsimd.dma_start(brd128[:], brd_d[:].broadcast_to((P, 1)))

            ident = res.tile([P, P], bf16, tag="ident")
            masks.make_identity(nc, ident[:])

            # W^T stream: SP queue, one 512KB slab per k-tile (each trigger
            # occupies the queue ~0.6us, so fewer/larger transfers win).
            w_sb = res.tile([P, KT * O], bf16, tag="w_sb")
            for k in range(KT):
                nc.sync.dma_start(w_sb[:, k * O:(k + 1) * O],
                                  wt_d[k * P:(k + 1) * P, :])

            # --- main loop: one 128-token tile at a time.
            # LoRA finalize for tile t is emitted after tile t's main sweep;
            # the h->sigmoid->g->g^T chain latency is mostly hidden because
            # the h matmul leads each k-group and the chain engines (ACT/DVE)
            # are otherwise idle.
            for t in range(NT):
                if 3 <= t < NT - 3:
                    load_x(t + 3)

                accs = [
                    psA.tile([P, 512], f32, tag="acc", name=f"acc_{t}_{j}")
                    for j in range(4)
                ]
                h = psH.tile([P, HN], f32, tag="h", name=f"h_{t}")
                gt4 = gpool.tile([P, P], bf16, tag="gt4", name=f"gt4_{t}")
                for k in range(KT):
                    lhs = xts[t][:, k * P:(k + 1) * P]
                    # h matmuls front-loaded 2-per-iteration so the routing
                    # chain (sigmoid -> g -> g^T -> gt copies) finishes while
                    # the main sweep still streams.
                    if k < 8:
                        for c in (2 * k, 2 * k + 1):
                            nc.tensor.matmul(
                                h[:], xts[t][:, c * P:(c + 1) * P],
                                aat_sb[:, c * HN:(c + 1) * HN],
                                start=(c == 0), stop=(c == KT - 1))
                    for j in range(4):
                        nc.tensor.matmul(
                            accs[j][:], lhs,
                            w_sb[:, k * O + j * 512:k * O + (j + 1) * 512],
                            start=(k == 0), stop=False,
                        )
                    # HAM warmers: while the PE trails the W^T DMA stream in
                    # tile 0 it would idle between k-groups and re-throttle to
                    # half clock; re-reading the just-arrived slab keeps it
                    # warm and costs nothing once W is resident.
                    if t == 0:
                        wdum = psT.tile([P, 512], f32, tag="tstage",
                                        name=f"wd_{k}")
                        nc.tensor.matmul(wdum[:], lhs,
                                         w_sb[:, k * O:k * O + 512],
                                         start=True, stop=True)
                        if k == 0:
                            load_x(1)
                        elif k == 2:
                            load_x(2)
                        elif k == 4:
                            nc.scalar.dma_start(btx_sb[:], btx_d[:])
                        elif k == 6:
                            load_x(3)
                    if t == 1 and k == 0:
                        load_x(4)
                    if t == 2 and k == 0:
                        load_x(5)
                    if k == 7:
                        # routing: w1 = sigmoid(dlogit + brd), w0 = 1 - w1,
                        # both scaled by alpha/rank
                        srow = small.tile([P, 1], f32, tag="srow",
                                          name=f"srow_{t}")
                        nc.scalar.activation(
                            srow[:], h[:, ER:ER + 1],
                            mybir.ActivationFunctionType.Sigmoid,
                            bias=brd128[:, 0:1], scale=1.0)
                        w1s = small.tile([P, 1], f32, tag="w1s", name=f"w1s_{t}")
                        nc.vector.tensor_scalar_mul(w1s[:], srow[:], SCALE)
                        w0s = small.tile([P, 1], f32, tag="w0s", name=f"w0s_{t}")
                        nc.vector.tensor_scalar(
                            w0s[:], srow[:], -SCALE, SCALE,
                            mybir.AluOpType.mult, mybir.AluOpType.add)
                        g = gpool.tile([P, GK], bf16, tag="g", name=f"g_{t}")
                        nc.vector.tensor_scalar_mul(g[:, 0:8], h[:, 0:8], w0s[:])
                        nc.vector.tensor_scalar_mul(g[:, 8:16], h[:, 8:16], w1s[:])
                        nc.vector.memset(g[:, 16:17], 1.0)
                        nc.vector.memset(g[:, 17:18], 0.0)
                    if k == 11:
                        # g^T on the PE, then replicate into 4 row strips so
                        # the finalize matmuls can run on 4 concurrent 32-row
                        # tiles of the array.
                        gst = psT.tile([GK, P], bf16, tag="tstage",
                                       name=f"gst_{t}")
                        nc.tensor.transpose(gst[:], g[:], ident[:])
                        for jj in range(4):
                            nc.scalar.copy(gt4[32 * jj:32 * jj + GK, :], gst[:])

                # LoRA finalize + bias: 4 concurrent row-tiled K=18 matmuls
                for jj in range(4):
                    nc.tensor.matmul(accs[jj][:],
                                     gt4[32 * jj:32 * jj + GK, :],
                                     btx_sb[32 * jj:32 * jj + GK, :],
                                     start=False, stop=True,
                                     tile_position=(32 * jj, 0))

                # drain: bias already folded in via the ones-row; pure copy,
                # split across Vector and Scalar engines.
                outt = opool.tile([P, O], bf16, tag="outt", name=f"out_{t}")
                for j in range(4):
                    if j < 2:
                        nc.vector.tensor_copy(outt[:, j * 512:(j + 1) * 512],
                                              accs[j][:])
                    else:
                        nc.scalar.copy(outt[:, j * 512:(j + 1) * 512],
                                       accs[j][:])
                    if j % 2 == 1:
                        nc.sync.dma_start(
                            out_d[t * P:(t + 1) * P, (j - 1) * 512:(j + 1) * 512],
                            outt[:, (j - 1) * 512:(j + 1) * 512])

    nc.compile()
    return nc


def _prep_host(x, W_base, b_base, A, B, W_router, b_router):
    """Host-side layout prep + sharding. Returns per-core input maps."""
    x_flat = np.ascontiguousarray(np.asarray(x, dtype=np.float32).reshape(-1, D))
    xb = x_flat.astype(BF16)
    # per-core pre-transposed tiling: [core, t, p(d within k), k*128+j(tok)]
    xt_all = np.ascontiguousarray(
        xb.reshape(NCORES, NT, P, KT, P).transpose(0, 1, 4, 3, 2)
    ).reshape(NCORES, NT * P, KT * P)

    wt = np.ascontiguousarray(np.asarray(W_base, dtype=np.float32).T).astype(BF16)

    a_cat = np.asarray(A, dtype=np.float32).reshape(ER, D)          # [16, D]
    aat = np.zeros((D, HN), dtype=np.float32)
    aat[:, :ER] = a_cat.T
    wr = np.asarray(W_router, dtype=np.float32)
    aat[:, ER] = wr[:, 1] - wr[:, 0]
    # pre-arrange for contiguous per-partition DMA: [P, KT*HN]
    aat = np.ascontiguousarray(
        aat.reshape(KT, P, HN).transpose(1, 0, 2).reshape(P, KT * HN)
    ).astype(BF16)

    b_cat = np.concatenate([np.asarray(B, dtype=np.float32)[0],
                            np.asarray(B, dtype=np.float32)[1]], axis=1)  # [O, 16]
    btx_full = np.zeros((GK, O), dtype=np.float32)
    btx_full[:ER] = b_cat.T
    btx_full[ER] = np.asarray(b_base, dtype=np.float32)  # ones-row adds bias
    # 4 row strips at partition offsets 0/32/64/96, one 512-col chunk each,
    # so the finalize matmuls run on 4 concurrent 32-row PE tiles
    btx = np.zeros((P, 512), dtype=np.float32)
    for jj in range(4):
        btx[32 * jj:32 * jj + GK] = btx_full[:, jj * 512:(jj + 1) * 512]
    btx = np.ascontiguousarray(btx).astype(BF16)

    brd = np.array([[np.float32(b_router[1]) - np.float32(b_router[0])]],
                   dtype=np.float32)

    in_maps = []
    for c in range(NCORES):
        in_maps.append({
            "xt": xt_all[c],
            "wt": wt,
            "aat": aat,
            "btx": btx,
            "brd": brd,
        })
    return in_maps


def kernel(x, W_base, b_base, A, B, W_router, b_router):
    from concourse import bass_utils

    # NOTE: walrus's LDWEIGHTS-dedup (--enable-ldw-opt=true) rejects bf16
    # weight loads ("InstLdweights is not compatible with LDW optimization")
    # because they take the FWL path; bf16 LDWs are ~2x faster than fp32 and
    # hidden by the PE's pull-ahead window, so the dedup isn't needed.
    if "nc" not in _CACHE:
        _CACHE["nc"] = _build()
    nc = _CACHE["nc"]

    in_maps = _prep_host(x, W_base, b_base, A, B, W_router, b_router)
    res = None
    for attempt in range(3):
        try:
            res = bass_utils.run_bass_kernel_spmd(
                nc, in_maps, core_ids=list(range(NCORES)))
            break
        except Exception:
            # rare transient NRT_EXEC_UNIT_UNRECOVERABLE observed once;
            # the same NEFF runs fine on retry
            if attempt == 2:
                raise
    out = np.concatenate(
        [np.asarray(res.results[c]["out"]).astype(np.float32)
         for c in range(NCORES)], axis=0)
    return out.reshape(np.asarray(x).shape[0], -1, O)
